# revision 19
# baseline (speedup 1.0000x reference)
"""BayesGNN (2x GCNConv + mean-pool + MLP head) on 8 Trainium2 NeuronCores.

Strategy (dst-node sharding, gather-based message passing, bf16):
  * Nodes are permuted host-side: sorted by degree and dealt round-robin to
    the 8 cores, so every core/bucket sees a near-identical degree profile
    (shrinks the max-over-cores slot envelope that all cores must pad to).
  * Symmetric GCN normalization is factorized: the x table is prescaled by
    deg^-1/2 host-side, h1 is stored prescaled by deg^-1/2 (folded into the
    ReLU's per-partition scale), so no per-edge norm multiplies remain.
  * Edges (+ self loops) are bucketed by (owner core, dst block of 128, src
    bank of 25000). Buckets are packed back-to-back within each
    (8-block quad, bank) gather op, padded to 128 only per op. Chunks that
    straddle two blocks run one selection matmul per block, disambiguated
    by a block-parity offset (+128) baked into the dst metadata.
  * conv aggregation per 128-edge chunk: dma_gather rows (bf16, 256B/512B)
    + Sel[slot, d] = (dstq[slot] == iota) built on the DVE, then PSUM
    accumulation via TensorE: agg[feat, dst] += Gchunk^T @ Sel.
  * conv1 epilogue: h1' = relu(deg^-1 * (agg^T @ W1)) in bf16 via the ACT
    engine's per-partition scale; conv2 aggregates gathered h1' rows in two
    128-feature halves (no transposes needed for the @W2).
  * h1' shards are AllGathered (bf16, Shared scratchpad) so every core holds
    the full table for conv2's gathers.
  * Mean-pool partials ([64,257] incl counts) accumulate in PSUM per quad,
    are AllReduced once, and the tiny MLP head runs redundantly per core.
"""

import os
import numpy as np
import ml_dtypes

import concourse.bass as bass
import concourse.bacc as bacc
import concourse.mybir as mybir
import concourse.tile as tile
from concourse.masks import make_identity

F32 = mybir.dt.float32
BF16 = mybir.dt.bfloat16
I16 = mybir.dt.int16
I32 = mybir.dt.int32
AF = mybir.ActivationFunctionType
OP = mybir.AluOpType
NPBF = ml_dtypes.bfloat16

BLK = 128  # dst nodes per block (PSUM partition count)
DEAD = 384.0  # dst sentinel for filler slots (matches no iota value, exact bf16)


class Dims:
    def __init__(self, N, DIN, HID, NG, NCLS, NCORES=8, BANKS=4, QUAD=4):
        assert DIN == 128 and HID == 256
        self.N, self.DIN, self.HID = N, DIN, HID
        self.NG, self.NCLS = NG, NCLS
        self.NCORES, self.BANKS, self.QUAD = NCORES, BANKS, QUAD
        assert N % NCORES == 0
        self.NPC = N // NCORES
        assert N % BANKS == 0
        self.BROWS = N // BANKS
        assert self.BROWS <= 32767, "bank rows must fit int16 index"
        self.NBLK = (self.NPC + BLK - 1) // BLK
        self.LAST_ROWS = self.NPC - (self.NBLK - 1) * BLK
        self.quads = [
            list(range(i, min(i + QUAD, self.NBLK)))
            for i in range(0, self.NBLK, QUAD)
        ]


DIMS = Dims(N=100000, DIN=128, HID=256, NG=64, NCLS=10)


class Structure:
    """Packed slot layout + per-chunk block plans, identical across cores."""

    def __init__(self, dims: Dims, cmax: np.ndarray):
        d = dims
        self.cmax = cmax  # [NBLK, BANKS] per-bucket envelope (max over cores)
        self.slot_off = np.zeros((d.NBLK, d.BANKS), np.int64)
        self.op_off = {}
        self.op_len = {}
        off = 0
        for qi, q in enumerate(d.quads):
            for b in range(d.BANKS):
                self.op_off[(qi, b)] = off
                for blk in q:
                    self.slot_off[blk, b] = off
                    off += int(cmax[blk, b])
                pad = (-(off - self.op_off[(qi, b)])) % BLK
                off += pad
                self.op_len[(qi, b)] = off - self.op_off[(qi, b)]
        self.TOT = off
        assert self.TOT % BLK == 0

        # chunk plans: for each (qi, bank) op, for each 128-slot chunk, the
        # list of blocks whose bucket overlaps the chunk
        self.plan = {}
        self.touches = np.zeros(d.NBLK, np.int64)
        for qi, q in enumerate(d.quads):
            for b in range(d.BANKS):
                o0 = self.op_off[(qi, b)]
                chunks = []
                for j in range(self.op_len[(qi, b)] // BLK):
                    lo, hi = o0 + j * BLK, o0 + (j + 1) * BLK
                    blks = [
                        blk
                        for blk in q
                        if self.slot_off[blk, b] < hi
                        and self.slot_off[blk, b] + self.cmax[blk, b] > lo
                    ]
                    assert len(blks) <= 2, "chunk spans >2 blocks"
                    chunks.append(blks)
                    for blk in blks:
                        self.touches[blk] += 1
                self.plan[(qi, b)] = chunks
        assert (self.touches > 0).all()


def _preprocess(dims: Dims, x, edge_index, batch):
    """Host-side: degree-dealt node permutation, prescaled bf16 x table,
    packed bucket layout, per-core gather metadata."""
    d = dims
    x = np.asarray(x, np.float32)
    src = np.asarray(edge_index[0], np.int64)
    dst = np.asarray(edge_index[1], np.int64)
    batch = np.asarray(batch, np.int64)

    deg = 1.0 + np.bincount(dst, minlength=d.N).astype(np.float64)
    dinv = 1.0 / np.sqrt(deg)

    # permutation: sort nodes by degree (desc), deal round-robin to cores
    order = np.argsort(-deg, kind="stable")
    rank = np.empty(d.N, np.int64)
    rank[order] = np.arange(d.N)
    phys = (rank % d.NCORES) * d.NPC + rank // d.NCORES  # node -> physical slot

    xb = np.empty((d.N, d.DIN), NPBF)
    xb[phys] = (x * dinv[:, None].astype(np.float32)).astype(NPBF)
    batch_p = np.empty(d.N, np.int64)
    batch_p[phys] = batch
    dinv_p = np.empty(d.N, np.float64)
    dinv_p[phys] = dinv

    loops = np.arange(d.N, dtype=np.int64)
    s2 = np.concatenate([phys[src], loops])
    d2 = np.concatenate([phys[dst], loops])

    core = d2 // d.NPC
    local = d2 - core * d.NPC
    blk = local // BLK
    dstloc = (local - blk * BLK) + 128 * (blk % 2)  # parity-coded local dst
    bank = s2 // d.BROWS
    idx16 = (s2 - bank * d.BROWS).astype(np.int16)

    key = (core * d.NBLK + blk) * d.BANKS + bank
    eorder = np.lexsort((s2, key))
    key_s = key[eorder]
    idx16_s = idx16[eorder]
    dstloc_s = dstloc[eorder].astype(np.float32)

    nkeys = d.NCORES * d.NBLK * d.BANKS
    counts = np.bincount(key_s, minlength=nkeys).reshape(
        d.NCORES, d.NBLK, d.BANKS
    )
    st = Structure(d, counts.max(axis=0))
    starts = np.concatenate([[0], np.cumsum(counts.reshape(-1))])

    per_core = []
    for c in range(d.NCORES):
        idx_all = np.zeros(st.TOT, np.int16)
        dst_all = np.full(st.TOT, DEAD, np.float32)
        for b_ in range(d.NBLK):
            for bk in range(d.BANKS):
                k = (c * d.NBLK + b_) * d.BANKS + bk
                s0, s1 = starts[k], starts[k + 1]
                n = s1 - s0
                if n == 0:
                    continue
                o = st.slot_off[b_, bk]
                idx_all[o : o + n] = idx16_s[s0:s1]
                dst_all[o : o + n] = dstloc_s[s0:s1]
        gidx = np.tile(
            np.ascontiguousarray(idx_all.reshape(-1, 16).T), (8, 1)
        )  # [128, TOT/16]
        gdst = np.ascontiguousarray(
            dst_all.reshape(-1, BLK).T
        )  # [128, TOT/128] f32 (is_equal scalar operands must be f32)

        nodes = batch_p[c * d.NPC : (c + 1) * d.NPC].astype(np.float32)
        bl = np.full((BLK, d.NBLK), float(d.NG), np.float32)
        dv = dinv_p[c * d.NPC : (c + 1) * d.NPC]
        d2c = np.ones((BLK, d.NBLK), np.float32)
        d1c = np.ones((BLK, d.NBLK), np.float32)
        for b_ in range(d.NBLK):
            rows = BLK if b_ < d.NBLK - 1 else d.LAST_ROWS
            bl[:rows, b_] = nodes[b_ * BLK : b_ * BLK + rows]
            d2c[:rows, b_] = (dv[b_ * BLK : b_ * BLK + rows] ** 2).astype(
                np.float32
            )
            d1c[:rows, b_] = dv[b_ * BLK : b_ * BLK + rows].astype(np.float32)
        per_core.append(
            {
                "gidx": gidx,
                "gdst": gdst,
                "bloc": bl,
                "dinv2c": d2c,
                "dinv1c": d1c,
                "sqdr": (1.0 / dv).astype(np.float32).reshape(1, d.NPC),
            }
        )
    return st, per_core, xb


def _build(tc, aps, dims: Dims, st: Structure, has_b1, has_b2):
    d = dims
    nc = tc.nc
    HID, DIN, NG, NCLS = d.HID, d.DIN, d.NG, d.NCLS
    rg = [list(range(d.NCORES))]
    nocoll = bool(os.environ.get("KERNEL_NOCOLL"))

    import contextlib

    with contextlib.ExitStack() as ctx:
        sp = ctx.enter_context(tc.tile_pool(name="sbuf", bufs=1))
        pp = ctx.enter_context(tc.tile_pool(name="psum", bufs=1, space="PSUM"))
        dp = ctx.enter_context(tc.tile_pool(name="dram", bufs=1, space="DRAM"))

        # ---- persistent DRAM tiles -------------------------------------
        h1shard = dp.tile([d.NPC, HID], BF16, name="h1shard")
        arin = dp.tile([NG, HID + 1], F32, name="arin")
        arout = dp.tile([NG, HID + 1], F32, name="arout")

        # ---- persistent SBUF constants (weights cast to bf16 on DVE) ---
        def load_bf16(name, src_ap, rows, cols, split):
            t32 = sp.tile([128, cols * split], F32, name=name + "32")
            for h in range(split):
                nc.sync.dma_start(
                    out=t32[:, h * cols : (h + 1) * cols],
                    in_=src_ap[h * 128 : (h + 1) * 128, :],
                )
            tb = sp.tile([128, cols * split], BF16, name=name)
            nc.vector.tensor_copy(out=tb[:], in_=t32[:])
            return tb

        w1_sb = load_bf16("w1_sb", aps["W1"], 128, HID, 1)
        w2_sb = load_bf16("w2_sb", aps["W2"], 128, HID, 2)
        wf1_sb = sp.tile([128, 2 * HID], F32, name="wf1_sb")
        wf2_sb = sp.tile([128, 2 * NCLS], F32, name="wf2_sb")
        for h in range(2):
            nc.sync.dma_start(
                out=wf1_sb[:, h * HID : (h + 1) * HID],
                in_=aps["Wf1"][h * 128 : (h + 1) * 128, :],
            )
            nc.sync.dma_start(
                out=wf2_sb[:, h * NCLS : (h + 1) * NCLS],
                in_=aps["Wf2"][h * 128 : (h + 1) * 128, :],
            )

        b1_sb = sp.tile([1, HID], F32, name="b1_sb")
        b2_sb = sp.tile([1, HID], F32, name="b2_sb")
        bf1_sb = sp.tile([1, HID], F32, name="bf1_sb")
        bf2_sb = sp.tile([1, NCLS], F32, name="bf2_sb")
        nc.sync.dma_start(out=b1_sb[:], in_=aps["b1"][:, :])
        nc.sync.dma_start(out=b2_sb[:], in_=aps["b2"][:, :])
        nc.sync.dma_start(out=bf1_sb[:], in_=aps["bf1"][:, :])
        nc.sync.dma_start(out=bf2_sb[:], in_=aps["bf2"][:, :])
        ident = sp.tile([128, 128], F32, name="ident")
        make_identity(nc, ident[:])
        ident_b = sp.tile([128, 128], BF16, name="ident_b")
        make_identity(nc, ident_b[:])
        iota_i = sp.tile([128, 256], I32, name="iota_i")
        nc.gpsimd.iota(iota_i[:], pattern=[[1, 256]], base=0, channel_multiplier=0)
        iota_b = sp.tile([128, 256], BF16, name="iota_b")
        nc.vector.tensor_copy(out=iota_b[:], in_=iota_i[:])
        ones1 = sp.tile([1, 128], F32, name="ones1")
        nc.vector.memset(ones1[:], 1.0)
        bloc_sb = sp.tile([BLK, d.NBLK], F32, name="bloc_sb")
        nc.sync.dma_start(out=bloc_sb[:], in_=aps["bloc"][:, :])
        dinv2_sb = sp.tile([BLK, d.NBLK], F32, name="dinv2_sb")
        nc.sync.dma_start(out=dinv2_sb[:], in_=aps["dinv2c"][:, :])
        dinv1_sb = sp.tile([BLK, d.NBLK], F32, name="dinv1_sb")
        nc.sync.dma_start(out=dinv1_sb[:], in_=aps["dinv1c"][:, :])
        # sqrt(deg) row for rank-1 bias injection (b / dinv per local node)
        sqdr_sb = None
        if has_b1 or has_b2:
            sqdr_sb = sp.tile([1, d.NPC], F32, name="sqdr_sb")
            nc.sync.dma_start(out=sqdr_sb[:], in_=aps["sqdr"][:, :])

        pooled_acc = sp.tile([NG, HID + 1], F32, name="pooled_acc")
        nc.vector.memset(pooled_acc[:], 0.0)

        def emit_conv(conv_idx, table_ap, ELEM):
            """Gather + aggregate + epilogue for one conv layer.

            PSUM accumulation groups claim a whole 2KB bank (zero region), so
            each block gets exactly one group: conv1 agg[feat, dst] =
            Gchunk^T @ Sel; conv2 agg[dst, feat256] = Sel^T @ Gchunk.
            """
            first = {blk: True for blk in range(d.NBLK)}
            done = np.zeros(d.NBLK, np.int64)
            for qi, q in enumerate(d.quads):
                gt = {}
                dstq = {}
                for b in range(d.BANKS):
                    oplen = st.op_len[(qi, b)]
                    o0 = st.op_off[(qi, b)]
                    nch = oplen // BLK
                    idxq = sp.tile(
                        [128, oplen // 16], I16, tag="idxq", bufs=3, name="idxq"
                    )
                    nc.sync.dma_start(
                        out=idxq[:],
                        in_=aps["gidx"][:, o0 // 16 : (o0 + oplen) // 16],
                    )
                    dq = sp.tile([128, nch], F32, tag="dstq", bufs=3, name="dstq")
                    nc.sync.dma_start(
                        out=dq[:], in_=aps["gdst"][:, o0 // BLK : o0 // BLK + nch]
                    )
                    dstq[b] = dq
                    g = sp.tile(
                        [128, nch * ELEM],
                        BF16,
                        tag=f"g{conv_idx}",
                        bufs=3,
                        name=f"g{conv_idx}t",
                    )
                    gv = g.rearrange("p (g e) -> p g e", e=ELEM)
                    if os.environ.get("KERNEL_NOGATHER"):
                        nc.vector.memset(g[:], 0.5)
                    else:
                        nc.gpsimd.dma_gather(
                            gv,
                            table_ap[b * d.BROWS : (b + 1) * d.BROWS, :],
                            idxq[:],
                            oplen,
                            oplen,
                            ELEM,
                            single_packet=False,
                            queue_num=b,
                        )
                    gt[b] = g

                # one accumulation group (= one whole PSUM bank) per block
                agg = {
                    blk: pp.tile([128, ELEM], F32, tag="agg", bufs=4, name="agg")
                    for blk in q
                }

                for b in range(d.BANKS):
                    for j, blks in enumerate(st.plan[(qi, b)]):
                        if not blks:
                            continue
                        gch = gt[b][:, j * ELEM : (j + 1) * ELEM]
                        for blk in blks:
                            sel = sp.tile(
                                [128, 128], BF16, tag="sel", bufs=8, name="sel"
                            )
                            par = (blk % 2) * 128
                            nc.vector.tensor_scalar(
                                out=sel[:],
                                in0=iota_b[:, par : par + 128],
                                scalar1=dstq[b][:, j : j + 1],
                                scalar2=None,
                                op0=OP.is_equal,
                            )
                            last = done[blk] == st.touches[blk] - 1
                            if conv_idx == 1:
                                nc.tensor.matmul(
                                    out=agg[blk][:],
                                    lhsT=gch,
                                    rhs=sel[:],
                                    start=first[blk],
                                    stop=last,
                                )
                            else:
                                nc.tensor.matmul(
                                    out=agg[blk][:],
                                    lhsT=sel[:],
                                    rhs=gch,
                                    start=first[blk],
                                    stop=last,
                                )
                            first[blk] = False
                            done[blk] += 1

                # ---- per-block epilogue ---------------------------------
                poolq = None
                for bi, blk in enumerate(q):
                    assert done[blk] == st.touches[blk]
                    rows = BLK if blk < d.NBLK - 1 else d.LAST_ROWS
                    w_sb, bias_sb, has_bias = (
                        (w1_sb, b1_sb, has_b1)
                        if conv_idx == 1
                        else (w2_sb, b2_sb, has_b2)
                    )
                    if conv_idx == 1:
                        # agg = [feat, dst]: pre[dst, HID] = agg^T @ W1
                        a1 = sp.tile(
                            [128, 128], BF16, tag="aggsb", bufs=6, name="aggsb"
                        )
                        nc.scalar.activation(a1[:], agg[blk][:], AF.Copy)
                        lhsts = [a1[:]]
                    else:
                        # agg = [dst, feat256]: transpose to [feat, dst] halves
                        a2 = sp.tile(
                            [128, HID], BF16, tag="aggsb", bufs=6, name="aggsb"
                        )
                        nc.scalar.activation(a2[:], agg[blk][:], AF.Copy)
                        t2 = pp.tile([128, HID], BF16, tag="tp", bufs=1, name="t2")
                        nc.tensor.transpose(t2[:, :128], a2[:, :128], ident_b[:])
                        nc.tensor.transpose(t2[:, 128:], a2[:, 128:], ident_b[:])
                        t2sb = sp.tile(
                            [128, HID], BF16, tag="t2sb", bufs=2, name="t2sb"
                        )
                        nc.vector.tensor_copy(out=t2sb[:], in_=t2[:])
                        lhsts = [t2sb[:, :128], t2sb[:, 128:]]
                    pre = pp.tile([128, HID], F32, tag="mm", bufs=2, name="pre")
                    for h, lh in enumerate(lhsts):
                        nc.tensor.matmul(
                            out=pre[:],
                            lhsT=lh,
                            rhs=w_sb[:, h * HID : (h + 1) * HID],
                            start=(h == 0),
                            stop=(h == len(lhsts) - 1) and not has_bias,
                        )
                    if has_bias:
                        nc.tensor.matmul(
                            out=pre[:],
                            lhsT=sqdr_sb[:, blk * BLK : blk * BLK + BLK],
                            rhs=bias_sb[:],
                            start=False,
                            stop=True,
                        )
                    scale = (dinv2_sb if conv_idx == 1 else dinv1_sb)[
                        :, blk : blk + 1
                    ]
                    if conv_idx == 1:
                        ht = sp.tile([128, HID], BF16, tag="hsb", bufs=4, name="ht")
                        nc.scalar.activation(ht[:], pre[:], AF.Relu, scale=scale)
                        nc.sync.dma_start(
                            out=h1shard[blk * BLK : blk * BLK + rows, :],
                            in_=ht[:rows, :],
                        )
                    else:
                        # 257th column = ones so the pool matmul also counts
                        ht = sp.tile(
                            [128, HID + 1], BF16, tag="hsb", bufs=4, name="ht"
                        )
                        nc.scalar.activation(
                            ht[:, :HID], pre[:], AF.Relu, scale=scale
                        )
                        nc.vector.memset(ht[:, HID : HID + 1], 1.0)
                        bsel = sp.tile(
                            [128, NG], BF16, tag="bsel", bufs=4, name="bsel"
                        )
                        nc.vector.tensor_scalar(
                            out=bsel[:],
                            in0=iota_b[:, :NG],
                            scalar1=bloc_sb[:, blk : blk + 1],
                            scalar2=None,
                            op0=OP.is_equal,
                        )
                        if poolq is None:
                            poolq = pp.tile(
                                [NG, HID + 1], F32, tag="pool", bufs=1, name="poolq"
                            )
                        nc.tensor.matmul(
                            out=poolq[:],
                            lhsT=bsel[:],
                            rhs=ht[:],
                            start=(bi == 0),
                            stop=(bi == len(q) - 1),
                        )
                if conv_idx == 2:
                    nc.vector.tensor_tensor(
                        out=pooled_acc[:],
                        in0=pooled_acc[:],
                        in1=poolq[:],
                        op=OP.add,
                    )

        phase = os.environ.get("KERNEL_PHASE", "full")
        reps = int(os.environ.get("KERNEL_BENCH_REPEAT", "1"))

        def emit_body(rep):
            # Shared scratchpad allows a single writer instruction, so each
            # rep gets its own AllGather output tensor.
            h1full = dp.tile(
                [d.N, HID],
                BF16,
                name=f"h1full{rep}",
                addr_space="Local" if nocoll else "Shared",
            )
            # ---- conv1 -------------------------------------------------
            emit_conv(1, aps["xb"], DIN)
            if phase == "conv1":
                return

            # ---- AllGather h1' -----------------------------------------
            if phase != "noag":
                if nocoll:
                    for k in range(d.NCORES):
                        nc.sync.dma_start(
                            out=h1full[k * d.NPC : (k + 1) * d.NPC, :],
                            in_=h1shard[:, :],
                        )
                else:
                    nc.gpsimd.collective_compute(
                        "AllGather",
                        OP.bypass,
                        replica_groups=rg,
                        ins=[h1shard.opt()],
                        outs=[h1full.opt()],
                    )
            if phase == "ag":
                return

            # ---- conv2 + pooling partials ------------------------------
            emit_conv(2, h1full, HID)

            # ---- AllReduce pooled --------------------------------------
            nc.sync.dma_start(out=arin[:, :], in_=pooled_acc[:])
            if nocoll:
                nc.sync.dma_start(out=arout[:, :], in_=arin[:, :])
            else:
                nc.gpsimd.collective_compute(
                    "AllReduce",
                    OP.add,
                    replica_groups=rg,
                    ins=[arin.opt()],
                    outs=[arout.opt()],
                )
            pooled_sb = sp.tile([NG, HID + 1], F32, name="pooled_sb", tag="psb")
            nc.sync.dma_start(out=pooled_sb[:], in_=arout[:, :])

            # ---- MLP head (redundant on every core) --------------------
            cnt = sp.tile([NG, 1], F32, name="cnt", tag="cnt")
            nc.vector.tensor_scalar_max(cnt[:], pooled_sb[:, HID : HID + 1], 1.0)
            rec = sp.tile([NG, 1], F32, name="rec", tag="rec")
            nc.vector.reciprocal(rec[:], cnt[:])
            pm = sp.tile([NG, HID], F32, name="pm", tag="pm")
            nc.vector.tensor_scalar_mul(pm[:], pooled_sb[:, :HID], rec[:])

            tpm = pp.tile([128, 2 * NG], F32, tag="mm", bufs=2, name="tpm")
            nc.tensor.transpose(tpm[:, :NG], pm[:, :128], ident[:NG, :NG])
            nc.tensor.transpose(tpm[:, NG:], pm[:, 128:], ident[:NG, :NG])
            pmT = sp.tile([128, 2 * NG], F32, name="pmT", tag="pmT")
            nc.vector.tensor_copy(out=pmT[:], in_=tpm[:])

            zpre = pp.tile([NG, HID], F32, tag="mm", bufs=2, name="zpre")
            nc.tensor.matmul(
                out=zpre[:], lhsT=pmT[:, :NG], rhs=wf1_sb[:, :HID],
                start=True, stop=False,
            )
            nc.tensor.matmul(
                out=zpre[:], lhsT=pmT[:, NG:], rhs=wf1_sb[:, HID:],
                start=False, stop=False,
            )
            nc.tensor.matmul(
                out=zpre[:], lhsT=ones1[:, :NG], rhs=bf1_sb[:],
                start=False, stop=True,
            )
            z_sb = sp.tile([NG, HID], F32, name="z_sb", tag="z_sb")
            nc.scalar.activation(z_sb[:], zpre[:], AF.Relu)

            tz = pp.tile([128, 2 * NG], F32, tag="mm", bufs=2, name="tz")
            nc.tensor.transpose(tz[:, :NG], z_sb[:, :128], ident[:NG, :NG])
            nc.tensor.transpose(tz[:, NG:], z_sb[:, 128:], ident[:NG, :NG])
            tzsb = sp.tile([128, 2 * NG], F32, name="tzsb", tag="tzsb")
            nc.vector.tensor_copy(out=tzsb[:], in_=tz[:])

            apre = pp.tile([NG, NCLS], F32, tag="pool", bufs=1, name="apre")
            nc.tensor.matmul(
                out=apre[:], lhsT=tzsb[:, :NG], rhs=wf2_sb[:, :NCLS],
                start=True, stop=False,
            )
            nc.tensor.matmul(
                out=apre[:], lhsT=tzsb[:, NG:], rhs=wf2_sb[:, NCLS:],
                start=False, stop=False,
            )
            nc.tensor.matmul(
                out=apre[:], lhsT=ones1[:, :NG], rhs=bf2_sb[:],
                start=False, stop=True,
            )
            # softplus(x) = relu(x) + ln(1 + exp(-|x|))
            ab = sp.tile([NG, NCLS], F32, name="ab", tag="ab")
            nc.scalar.activation(ab[:], apre[:], AF.Abs)
            en = sp.tile([NG, NCLS], F32, name="en", tag="en")
            nc.scalar.activation(en[:], ab[:], AF.Exp, scale=-1.0)
            nc.vector.tensor_scalar_add(en[:], en[:], 1.0)
            ln_t = sp.tile([NG, NCLS], F32, name="ln_t", tag="ln_t")
            nc.scalar.activation(ln_t[:], en[:], AF.Ln)
            rx = sp.tile([NG, NCLS], F32, name="rx", tag="rx")
            nc.scalar.activation(rx[:], apre[:], AF.Relu)
            alpha_sb = sp.tile([NG, NCLS], F32, name="alpha_sb", tag="alpha_sb")
            nc.vector.tensor_tensor(
                out=alpha_sb[:], in0=ln_t[:], in1=rx[:], op=OP.add
            )
            nc.vector.tensor_scalar_add(alpha_sb[:], alpha_sb[:], 0.001)
            nc.sync.dma_start(out=aps["alpha"][:, :], in_=alpha_sb[:])

        for _rep in range(reps):
            if _rep > 0:
                nc.vector.memset(pooled_acc[:], 0.0)
            emit_body(_rep)
        if phase != "full":
            dummy = sp.tile([NG, NCLS], F32, name="dummy")
            nc.vector.memset(dummy[:], 1.0)
            nc.sync.dma_start(out=aps["alpha"][:, :], in_=dummy[:])


def build_module(dims: Dims, st: Structure, per_core0, xb, weights):
    nc = bacc.Bacc(
        "TRN2",
        target_bir_lowering=False,
        debug=False,
        enable_asserts=False,
        num_devices=dims.NCORES,
        num_swdge_queues=4,
    )
    aps = {}

    def inp(name, arr):
        aps[name] = nc.dram_tensor(
            name, list(arr.shape), mybir.dt.from_np(arr.dtype), kind="ExternalInput"
        ).ap()

    inp("xb", xb)
    for k, v in weights.items():
        inp(k, v)
    for k in ("gidx", "gdst", "bloc", "dinv2c", "dinv1c", "sqdr"):
        inp(k, per_core0[k])
    aps["alpha"] = nc.dram_tensor(
        "alpha", [dims.NG, dims.NCLS], F32, kind="ExternalOutput"
    ).ap()

    has_b1 = bool(np.any(weights["b1"] != 0))
    has_b2 = bool(np.any(weights["b2"] != 0))
    with tile.TileContext(nc) as tc:
        _build(tc, aps, dims, st, has_b1, has_b2)
    nc.compile()
    return nc


def _run(dims: Dims, st: Structure, per_core, xb, weights, trace=False):
    from concourse.bass_utils import run_bass_kernel_spmd

    d = dims
    nc = build_module(d, st, per_core[0], xb, weights)
    in_maps = []
    for c in range(d.NCORES):
        m = {"xb": xb, **weights, **per_core[c]}
        in_maps.append(m)
    res = run_bass_kernel_spmd(
        nc, in_maps, core_ids=list(range(d.NCORES)), trace=trace
    )
    return res


LAST_RESULT = None


def kernel(**inputs) -> np.ndarray:
    global LAST_RESULT
    d = DIMS
    st, per_core, xb = _preprocess(
        d, inputs["x"], inputs["edge_index"], inputs["batch"]
    )
    weights = {
        "W1": np.ascontiguousarray(np.asarray(inputs["W1"], np.float32)),
        "W2": np.ascontiguousarray(np.asarray(inputs["W2"], np.float32)),
        "Wf1": np.ascontiguousarray(np.asarray(inputs["Wf1"], np.float32)),
        "Wf2": np.ascontiguousarray(np.asarray(inputs["Wf2"], np.float32)),
        "b1": np.asarray(inputs["b1"], np.float32).reshape(1, -1),
        "b2": np.asarray(inputs["b2"], np.float32).reshape(1, -1),
        "bf1": np.asarray(inputs["bf1"], np.float32).reshape(1, -1),
        "bf2": np.asarray(inputs["bf2"], np.float32).reshape(1, -1),
    }
    trace = bool(os.environ.get("KERNEL_TRACE"))
    res = _run(d, st, per_core, xb, weights, trace=trace)
    LAST_RESULT = res
    return np.asarray(res.results[0]["alpha"])
